# revision 1
# baseline (speedup 1.0000x reference)
"""Trainium2 Bass kernel for a dense transformer block.

reference: x -> LN1 -> 16-head causal attention (+residual) -> LN2 -> MLP
(+residual), x: [2, 2048, 1024] fp32.

Sharding: sequence-parallel with zigzag load balancing, zero collectives.
Core c (of 8) handles batch c//4 and query chunks j=c%4 and 7-j (256 rows
each => 512 rows/core). Each core recomputes LN1+K/V for its batch's first
1792 rows (the longest strict prefix any chunk needs); the 512 own rows'
K/V/Q come from a separate fixed-position path so one uniform program runs
on all 8 cores (SPMD), with per-core behavior carried entirely by input
data (augmented score rows + additive masks).

Matmul layout trick: activations are kept feature-major (transposed via PE)
so every matmul's contraction dim sits on partitions; softmax denominators
ride the AV matmul as an appended ones-column of V; per-token divides use
gpsimd partition_broadcast. Matmuls run in float32r (TF32-like, 1 cyc/row
at moving-dim >= 256; stationary free dim must be 64 or 128; producers
must write float32r).
"""

import sys

sys.path.insert(0, "/opt/trn_rl_repo")

from contextlib import ExitStack

import numpy as np

import concourse.bacc as bacc
import concourse.mybir as mybir
import concourse.tile as tile
from concourse.bass_utils import run_bass_kernel_spmd
from concourse.masks import make_identity

F32 = mybir.dt.float32
MM = mybir.dt.float32r  # matmul operand dtype
AF = mybir.ActivationFunctionType
ALU = mybir.AluOpType

B, P, D, H, DH = 2, 2048, 1024, 16, 64
FF = 4 * D
EPS = 1e-5
NCORES = 8
KV = 1792            # rect-path kv rows (longest strict prefix = 7*256)
KC = KV // 128       # 14 rect kv chunks
QL = 512             # query rows per core
DC = D // 128        # 8 contraction chunks over D
FC = FF // 128       # 32 f-chunks
BIG = 30000.0        # additive mask magnitude; exp(-30000) == 0 in fp32

# N-tiling of the [*, KV] projection outputs (PSUM bank is 512 fp32 wide)
NT = [(0, 512), (512, 512), (1024, 512), (1536, 256)]


def _ln_tile(nc, spool, eps_t, src, dst):
    """dst = (src - mean(src)) * rsqrt(var(src) + EPS), rows on partitions."""
    stats = spool.tile([128, 2, nc.vector.BN_STATS_DIM], F32, tag="ln_stats")
    for sg in range(2):
        nc.vector.bn_stats(out=stats[:, sg, :], in_=src[:, sg * 512:(sg + 1) * 512])
    mv = spool.tile([128, nc.vector.BN_AGGR_DIM], F32, tag="ln_mv")
    nc.vector.bn_aggr(out=mv[:], in_=stats[:])
    rstd = spool.tile([128, 1], F32, tag="ln_rstd")
    nc.scalar.activation(out=rstd[:], in_=mv[:, 1:2], func=AF.Sqrt, bias=eps_t[:])
    nc.vector.reciprocal(out=rstd[:], in_=rstd[:])
    nc.vector.tensor_scalar(out=dst, in0=src, scalar1=mv[:, 0:1], scalar2=rstd[:],
                            op0=ALU.subtract, op1=ALU.mult)


def build_nc():
    nc = bacc.Bacc(trn_type="TRN2")

    xb = nc.declare_dram_parameter("xb", [KV, D], F32, isOutput=False)
    xq = nc.declare_dram_parameter("xq", [QL, D], F32, isOutput=False)
    wq = nc.declare_dram_parameter("wq", [D, D], MM, isOutput=False)
    wk = nc.declare_dram_parameter("wk", [D, D], MM, isOutput=False)
    wv = nc.declare_dram_parameter("wv", [D, D], MM, isOutput=False)
    bqkv = nc.declare_dram_parameter("bqkv", [3, D], F32, isOutput=False)
    wp = nc.declare_dram_parameter("wp", [D, D], MM, isOutput=False)
    w1 = nc.declare_dram_parameter("w1", [D, FF], MM, isOutput=False)
    w2 = nc.declare_dram_parameter("w2", [FF, D], MM, isOutput=False)
    b1v = nc.declare_dram_parameter("b1v", [FF], F32, isOutput=False)
    bpv = nc.declare_dram_parameter("bpv", [D], F32, isOutput=False)
    b2v = nc.declare_dram_parameter("b2v", [D], F32, isOutput=False)
    augq = nc.declare_dram_parameter("augq", [2, QL], MM, isOutput=False)
    augk = nc.declare_dram_parameter("augk", [2, KV], MM, isOutput=False)
    dmask = nc.declare_dram_parameter("dmask", [4, 128, 256], F32, isOutput=False)
    out = nc.declare_dram_parameter("out", [QL, D], F32, isOutput=True)

    # DRAM views reshaped for partition-major DMA
    wq_v = wq.ap().rearrange("(dc p) e -> p dc e", p=128)
    wk_v = wk.ap().rearrange("(dc p) e -> p dc e", p=128)
    wv_v = wv.ap().rearrange("(dc p) e -> p dc e", p=128)
    w1_v = w1.ap().rearrange("(dc p) f -> p dc f", p=128)
    bqkv_v = bqkv.ap().rearrange("b (h e) -> e b h", e=DH)      # [64, 3, 16]
    b1_v = b1v.ap().rearrange("(fc p) -> p fc", p=128)          # [128, 32]
    bp_v = bpv.ap().rearrange("(a d) -> a d", a=1)
    b2_v = b2v.ap().rearrange("(a d) -> a d", a=1)
    dm_v = dmask.ap().rearrange("c p n -> p c n")

    with tile.TileContext(nc) as tc, ExitStack() as ctx:
        persist = ctx.enter_context(tc.tile_pool(name="persist", bufs=1))
        trps = ctx.enter_context(tc.tile_pool(name="trps", bufs=2, space="PSUM"))
        spool = ctx.enter_context(tc.tile_pool(name="spool", bufs=3))

        # ---- constants
        ident = persist.tile([128, 128], F32)
        make_identity(nc, ident[:])
        eps_t = persist.tile([128, 1], F32)
        nc.vector.memset(eps_t[:], EPS)
        dm = persist.tile([128, 4, 256], F32)
        nc.sync.dma_start(dm[:], dm_v)
        bqkv_sb = persist.tile([64, 3, H], F32)
        nc.sync.dma_start(bqkv_sb[:], bqkv_v)
        b1_sb = persist.tile([128, FC], F32)
        nc.sync.dma_start(b1_sb[:], b1_v)
        bp_row = persist.tile([1, D], F32)
        nc.sync.dma_start(bp_row[:], bp_v)
        bp_bc = persist.tile([128, D], F32)
        nc.gpsimd.partition_broadcast(bp_bc[:], bp_row[0:1, :])
        b2_row = persist.tile([1, D], F32)
        nc.sync.dma_start(b2_row[:], b2_v)
        b2_bc = persist.tile([128, D], F32)
        nc.gpsimd.partition_broadcast(b2_bc[:], b2_row[0:1, :])

        oT = persist.tile([128, DC, QL], MM)   # attention out, feature-major

        def transpose_to(src, dst, pn=128):
            """dst[cols, rows] = src[rows, cols].T via PE (src [pn,128])."""
            tp = trps.tile([128, 128], F32, tag="tr")
            nc.tensor.transpose(tp[:, 0:pn], src, ident[0:pn, 0:pn])
            nc.vector.tensor_copy(dst, tp[:, 0:pn])

        with tc.tile_pool(name="ph2big", bufs=1) as ph2big:
            hT = ph2big.tile([128, DC, KV], MM)    # LN1(x_b[:1792]) transposed
            hqT = ph2big.tile([128, DC, QL], MM)   # LN1(x_own) transposed

            # ===== Phase 1: LN1 -> transposed activations =====
            with tc.tile_pool(name="xpool", bufs=3) as xpool, \
                 tc.tile_pool(name="hpool", bufs=2) as hpool:
                for i in range(KC):
                    xt = xpool.tile([128, D], F32, tag="xt")
                    nc.sync.dma_start(xt[:], xb.ap()[128 * i:128 * (i + 1), :])
                    ht = hpool.tile([128, D], F32, tag="ht")
                    _ln_tile(nc, spool, eps_t, xt[:], ht[:])
                    for dc in range(DC):
                        transpose_to(ht[:, 128 * dc:128 * (dc + 1)],
                                     hT[:, dc, 128 * i:128 * (i + 1)])
                for i in range(4):
                    xt = xpool.tile([128, D], F32, tag="xt")
                    nc.sync.dma_start(xt[:], xq.ap()[128 * i:128 * (i + 1), :])
                    ht = hpool.tile([128, D], F32, tag="ht")
                    _ln_tile(nc, spool, eps_t, xt[:], ht[:])
                    for dc in range(DC):
                        transpose_to(ht[:, 128 * dc:128 * (dc + 1)],
                                     hqT[:, dc, 128 * i:128 * (i + 1)])

            # ===== Phase 2: per-head QKV + attention =====
            with tc.tile_pool(name="wpool", bufs=2) as wpool, \
                 tc.tile_pool(name="kqpool", bufs=2) as kqpool, \
                 tc.tile_pool(name="vpool", bufs=2) as vpool, \
                 tc.tile_pool(name="apool", bufs=3) as apool, \
                 tc.tile_pool(name="rpool", bufs=2) as rpool, \
                 tc.tile_pool(name="qkvps", bufs=2, space="PSUM") as qkvps, \
                 tc.tile_pool(name="spsum", bufs=2, space="PSUM") as spsum, \
                 tc.tile_pool(name="opsum", bufs=1, space="PSUM") as opsum:

                for h in range(H):
                    wq_t = wpool.tile([128, DC, DH], MM, tag="wq_t")
                    nc.sync.dma_start(wq_t[:], wq_v[:, :, DH * h:DH * (h + 1)])
                    wk_t = wpool.tile([128, DC, DH], MM, tag="wk_t")
                    nc.sync.dma_start(wk_t[:], wk_v[:, :, DH * h:DH * (h + 1)])
                    wv_t = wpool.tile([128, DC, DH], MM, tag="wv_t")
                    nc.sync.dma_start(wv_t[:], wv_v[:, :, DH * h:DH * (h + 1)])

                    def proj(w_t, rhs_sb, n0, nl, ps_tag="qkv"):
                        ps = qkvps.tile([64, 512], F32, tag=ps_tag)
                        for dc in range(DC):
                            nc.tensor.matmul(ps[:, 0:nl], w_t[:, dc, :],
                                             rhs_sb[:, dc, n0:n0 + nl],
                                             start=(dc == 0), stop=(dc == DC - 1))
                        return ps

                    # q' [66, 512]: rows 0-63 q, rows 64-65 aug block indicators
                    qp = kqpool.tile([66, QL], MM, tag="qp")
                    ps = proj(wq_t, hqT, 0, QL)
                    nc.vector.tensor_scalar_add(qp[0:64, :], ps[:],
                                                bqkv_sb[:, 0, h:h + 1])
                    nc.sync.dma_start(qp[64:66, :], augq.ap())

                    # k' [66, 1792]: rows 64-65 aug prefix-validity rows
                    kp = kqpool.tile([66, KV], MM, tag="kp")
                    for n0, nl in NT:
                        ps = proj(wk_t, hT, n0, nl)
                        nc.vector.tensor_scalar_add(kp[0:64, n0:n0 + nl],
                                                    ps[:, 0:nl],
                                                    bqkv_sb[:, 1, h:h + 1])
                    nc.sync.dma_start(kp[64:66, :], augk.ap())

                    # v: project feature-major with an appended ones row
                    # (becomes col 64 = softmax denominator after transpose),
                    # retranspose chunks into [128, kc, 128] (cols 65+ zero).
                    vt = vpool.tile([65, KV], F32, tag="vt")
                    for n0, nl in NT:
                        ps = proj(wv_t, hT, n0, nl)
                        nc.vector.tensor_scalar_add(vt[0:64, n0:n0 + nl], ps[:, 0:nl],
                                                    bqkv_sb[:, 2, h:h + 1])
                    nc.vector.memset(vt[64:65, :], 1.0)
                    vp = vpool.tile([128, KC, 128], MM, tag="vp")
                    nc.vector.memset(vp[:, :, :].bitcast(F32), 0.0)
                    for kc in range(KC):
                        transpose_to(vt[:, 128 * kc:128 * (kc + 1)],
                                     vp[:, kc, 0:65], pn=65)

                    # diag-path k,v (own rows) from hqT
                    kd = kqpool.tile([64, QL], MM, tag="kd")
                    ps = proj(wk_t, hqT, 0, QL)
                    nc.vector.tensor_scalar_add(kd[:], ps[:], bqkv_sb[:, 1, h:h + 1])
                    vd = vpool.tile([65, QL], F32, tag="vd")
                    ps = proj(wv_t, hqT, 0, QL)
                    nc.vector.tensor_scalar_add(vd[0:64, :], ps[:], bqkv_sb[:, 2, h:h + 1])
                    nc.vector.memset(vd[64:65, :], 1.0)
                    vpd = vpool.tile([128, 4, 128], MM, tag="vpd")
                    nc.vector.memset(vpd[:, :, :].bitcast(F32), 0.0)
                    for ci in range(4):
                        transpose_to(vd[:, 128 * ci:128 * (ci + 1)],
                                     vpd[:, ci, 0:65], pn=65)

                    # ---- attention
                    # Two independent accumulators, one per query block's
                    # 256 columns (no PSUM subrange accumulation). Rect chunks
                    # >= 6 can never be valid for block A on any core
                    # (A-validity needs kc < 2j <= 6), so they only feed ops_b.
                    ops_a = opsum.tile([128, 256], F32, tag="ops_a")
                    ops_b = opsum.tile([128, 256], F32, tag="ops_b")
                    for kc in range(KC):
                        nw = QL if kc < 6 else 256
                        qo = 0 if kc < 6 else 256
                        sps = spsum.tile([128, nw], F32, tag="sps")
                        nc.tensor.matmul(sps[:], kp[:, 128 * kc:128 * (kc + 1)],
                                         qp[:, qo:qo + nw], start=True, stop=True)
                        att = apool.tile([128, nw], MM, tag="att")
                        nc.scalar.activation(att[:], sps[:], AF.Exp)
                        if kc < 6:
                            nc.tensor.matmul(ops_a[:], vp[:, kc, :],
                                             att[:, 0:256],
                                             start=(kc == 0), stop=False)
                            nc.tensor.matmul(ops_b[:], vp[:, kc, :],
                                             att[:, 256:512],
                                             start=(kc == 0), stop=False)
                        else:
                            nc.tensor.matmul(ops_b[:], vp[:, kc, :], att[:],
                                             start=False, stop=False)
                    for ci in range(4):
                        qoff = 256 * (ci // 2)
                        sps = spsum.tile([128, 256], F32, tag="sps")
                        nc.tensor.matmul(sps[:], kd[:, 128 * ci:128 * (ci + 1)],
                                         qp[0:64, qoff:qoff + 256],
                                         start=True, stop=True)
                        nc.vector.tensor_add(sps[:], sps[:], dm[:, ci, :])
                        att = apool.tile([128, 256], MM, tag="att")
                        nc.scalar.activation(att[:], sps[:], AF.Exp)
                        tgt = ops_a if ci < 2 else ops_b
                        nc.tensor.matmul(tgt[:], vpd[:, ci, :], att[:],
                                         start=False, stop=(ci in (1, 3)))

                    # normalize: o/s (s = ops row 64); odd heads land at
                    # partition 64 of oT, moved there by SBUF->SBUF DMA.
                    rec = rpool.tile([1, QL], F32, tag="rec")
                    nc.vector.reciprocal(rec[0:1, 0:256], ops_a[64:65, :])
                    nc.vector.reciprocal(rec[0:1, 256:512], ops_b[64:65, :])
                    sbc = rpool.tile([64, QL], F32, tag="sbc")
                    nc.gpsimd.partition_broadcast(sbc[:], rec[0:1, :])
                    prow = (h % 2) * 64
                    nc.vector.tensor_mul(oT[prow:prow + 64, h // 2, 0:256],
                                         ops_a[0:64, :], sbc[:, 0:256])
                    nc.vector.tensor_mul(oT[prow:prow + 64, h // 2, 256:512],
                                         ops_b[0:64, :], sbc[:, 256:512])

        # ===== Phase 3: attn_out = oT.T @ Wp; xmid = attn_out + xq + bp =====
        with tc.tile_pool(name="ph3big", bufs=1) as ph3big:
            xmid = ph3big.tile([128, 4, D], F32)
            with tc.tile_pool(name="wps", bufs=2) as wps, \
                 tc.tile_pool(name="xqp", bufs=2) as xqp, \
                 tc.tile_pool(name="finps", bufs=1, space="PSUM") as finps:
                for dh in range(2):
                    pss = [finps.tile([128, 512], F32, tag=f"fin{t}",
                                      name=f"fin{t}_{dh}")
                           for t in range(4)]
                    for dc in range(DC):
                        wpt = wps.tile([128, 512], MM, tag="wpt")
                        nc.sync.dma_start(
                            wpt[:],
                            wp.ap()[128 * dc:128 * (dc + 1), 512 * dh:512 * (dh + 1)])
                        for t in range(4):
                            nc.tensor.matmul(pss[t][:],
                                             oT[:, dc, 128 * t:128 * (t + 1)],
                                             wpt[:], start=(dc == 0),
                                             stop=(dc == DC - 1))
                    for t in range(4):
                        xqt = xqp.tile([128, 512], F32, tag="xqt")
                        nc.sync.dma_start(
                            xqt[:],
                            xq.ap()[128 * t:128 * (t + 1), 512 * dh:512 * (dh + 1)])
                        sl = xmid[:, t, 512 * dh:512 * (dh + 1)]
                        nc.vector.tensor_add(sl, pss[t][:], xqt[:])
                        nc.vector.tensor_add(sl, sl, bp_bc[:, 512 * dh:512 * (dh + 1)])

            # ===== Phase 4: LN2 -> h2T =====
            with tc.tile_pool(name="ph5big", bufs=1) as ph5big:
                h2T = ph5big.tile([128, DC, QL], MM)
                mT = ph5big.tile([128, FC, QL], MM)
                with tc.tile_pool(name="hpool2", bufs=2) as hpool2:
                    for i in range(4):
                        ht = hpool2.tile([128, D], F32, tag="h2t")
                        _ln_tile(nc, spool, eps_t, xmid[:, i, :], ht[:])
                        for dc in range(DC):
                            transpose_to(ht[:, 128 * dc:128 * (dc + 1)],
                                         h2T[:, dc, 128 * i:128 * (i + 1)])

                # ===== Phase 5: MLP + residual + output =====
                with tc.tile_pool(name="w1p", bufs=2) as w1p, \
                     tc.tile_pool(name="w2p", bufs=2) as w2p, \
                     tc.tile_pool(name="opool", bufs=3) as opool, \
                     tc.tile_pool(name="finps2", bufs=1, space="PSUM") as finps2, \
                     tc.tile_pool(name="mps", bufs=2, space="PSUM") as mps:
                    for dh in range(2):
                        pss = [finps2.tile([128, 512], F32, tag=f"fo{t}",
                                           name=f"fo{t}_{dh}")
                               for t in range(4)]
                        for fc in range(FC):
                            if dh == 0:
                                w1t = w1p.tile([128, DC, 128], MM, tag="w1t")
                                nc.sync.dma_start(
                                    w1t[:], w1_v[:, :, 128 * fc:128 * (fc + 1)])
                                mp = mps.tile([128, QL], F32, tag="mp")
                                for dc in range(DC):
                                    nc.tensor.matmul(mp[:], w1t[:, dc, :],
                                                     h2T[:, dc, :],
                                                     start=(dc == 0),
                                                     stop=(dc == DC - 1))
                                nc.scalar.activation(mT[:, fc, :], mp[:], AF.Gelu,
                                                     bias=b1_sb[:, fc:fc + 1])
                            w2t = w2p.tile([128, 512], MM, tag="w2t")
                            nc.sync.dma_start(
                                w2t[:],
                                w2.ap()[128 * fc:128 * (fc + 1),
                                        512 * dh:512 * (dh + 1)])
                            for t in range(4):
                                nc.tensor.matmul(pss[t][:],
                                                 mT[:, fc, 128 * t:128 * (t + 1)],
                                                 w2t[:], start=(fc == 0),
                                                 stop=(fc == FC - 1))
                        for t in range(4):
                            ot = opool.tile([128, 512], F32, tag="ot")
                            nc.vector.tensor_add(ot[:], pss[t][:],
                                                 xmid[:, t, 512 * dh:512 * (dh + 1)])
                            nc.vector.tensor_add(ot[:], ot[:],
                                                 b2_bc[:, 512 * dh:512 * (dh + 1)])
                            nc.sync.dma_start(
                                out.ap()[128 * t:128 * (t + 1),
                                         512 * dh:512 * (dh + 1)],
                                ot[:])

    nc.compile()
    return nc


_NC_CACHE = {}


def _get_nc():
    if "nc" not in _NC_CACHE:
        _NC_CACHE["nc"] = build_nc()
    return _NC_CACHE["nc"]


def _host_pack(inputs):
    x = np.ascontiguousarray(np.asarray(inputs["x"], dtype=np.float32))
    Wq = np.asarray(inputs["Wq"], np.float32).transpose(1, 0, 2).reshape(D, D)
    Wk = np.asarray(inputs["Wk"], np.float32).transpose(1, 0, 2).reshape(D, D)
    Wv = np.asarray(inputs["Wv"], np.float32).transpose(1, 0, 2).reshape(D, D)
    Wp = np.asarray(inputs["Wp"], np.float32)
    bp = np.asarray(inputs["bp"], np.float32)
    W1 = np.asarray(inputs["W1"], np.float32)
    b1 = np.asarray(inputs["b1"], np.float32)
    W2 = np.asarray(inputs["W2"], np.float32)
    b2 = np.asarray(inputs["b2"], np.float32)
    g1 = np.asarray(inputs["g1"], np.float32)
    be1 = np.asarray(inputs["be1"], np.float32)
    g2 = np.asarray(inputs["g2"], np.float32)
    be2 = np.asarray(inputs["be2"], np.float32)

    scale = np.float32(np.float64(D) ** -0.5)  # 1/32, exact power of two
    wq_p = np.ascontiguousarray(Wq * g1[:, None] * scale)
    wk_p = np.ascontiguousarray(Wk * g1[:, None])
    wv_p = np.ascontiguousarray(Wv * g1[:, None])
    # biases induced by the LN shift (be1), folded into q/k/v
    bqkv = np.stack([be1 @ Wq * scale, be1 @ Wk, be1 @ Wv]).astype(np.float32)
    w1_p = np.ascontiguousarray(W1 * g2[:, None])
    b1_p = (b1 + be2 @ W1).astype(np.float32)

    augq = np.zeros((2, QL), np.float32)
    augq[0, 0:256] = 1.0
    augq[1, 256:512] = 1.0

    # diag masks: additive, 0 keep / -BIG drop. Diag kv rows = own 512 rows
    # (A chunk then B chunk); columns = own 512 queries (A then B).
    dmask = np.empty((4, 128, 256), np.float32)
    ii = np.arange(128)
    jj = np.arange(256)
    for ci in range(4):
        loc = 128 * (ci % 2) + ii[:, None]          # row pos within the block
        keep = loc <= jj[None, :]                   # causal within block
        dmask[ci] = np.where(keep, 0.0, -BIG)

    shared = dict(wq=wq_p, wk=wk_p, wv=wv_p, bqkv=bqkv, wp=Wp, w1=w1_p,
                  b1v=b1_p, w2=W2, bpv=bp, b2v=b2, augq=augq,
                  dmask=np.ascontiguousarray(dmask))

    in_maps = []
    for c in range(NCORES):
        b, j = c // 4, c % 4
        xb_c = np.ascontiguousarray(x[b, :KV])
        xq_c = np.ascontiguousarray(
            np.concatenate([x[b, 256 * j:256 * (j + 1)],
                            x[b, 256 * (7 - j):256 * (8 - j)]], axis=0))
        augk = np.zeros((2, KV), np.float32)
        augk[0, 256 * j:] = -BIG        # block A valid rect prefix: t < 256j
        augk[1, 256 * (7 - j):] = -BIG  # block B valid rect prefix: t < 256(7-j)
        in_maps.append(dict(shared, xb=xb_c, xq=xq_c, augk=augk))
    return x, in_maps


def _unshard(results):
    out = np.empty((B, P, D), np.float32)
    for c in range(NCORES):
        b, j = c // 4, c % 4
        o = results[c]["out"]
        out[b, 256 * j:256 * (j + 1)] = o[0:256]
        out[b, 256 * (7 - j):256 * (8 - j)] = o[256:512]
    return out


def kernel(**inputs):
    x, in_maps = _host_pack(inputs)
    nc = _get_nc()
    res = run_bass_kernel_spmd(nc, in_maps, core_ids=list(range(NCORES)))
    return _unshard(res.results)



# revision 10
# speedup vs baseline: 1.5236x; 1.5236x over previous
"""Trainium2 Bass kernel for a dense transformer block.

reference: x -> LN1 -> 16-head causal attention (+residual) -> LN2 -> MLP
(+residual), x: [2, 2048, 1024] fp32.

Sharding (v3): tensor-parallel attention + sequence-parallel MLP.
Core c (of 8) computes heads {2c, 2c+1} over ALL 4096 tokens (both
batches): LN1 is replicated, QKV projection / scores / softmax / AV run
on the core's 2 heads only.  Two bf16 8-core AllToAlls (one per local
head) reshard the attention output o^T from head-major to
sequence-major; core c then owns tokens [512c, 512c+512) for the output
projection, LN2 and the MLP, and writes exactly those 512 rows.

All matmuls run in bf16 (1 PE cycle/row) with fp32 psum accumulation;
residuals are fp32.  fp8 was measured (numpy-emulated against the jax
oracle) to blow most of the 2e-2 error budget at any single site, so it
is not used.

Scheduling tricks: h^T comes from the XBAR DMA-transpose engine (no PE
transposes, no psum evacuations); LN1 emission is interleaved with the
QKV matmuls per 512-token group so the PE never starves; LN1 stats are
split between ACT (Copy/Square with free-dim accumulators) and DVE
(bn_stats) to balance engines; causal masks for the two diagonal
kv-chunks are injected into the scores psum by an identity-stationary
matmul on the PE; softmax denominators ride the AV matmul as an
all-ones column of V; the normalizer broadcast runs on gpsimd; exp
applies the D^-0.5 scale via the ACT scale field; the output projection
is split into two half-contractions so the first half overlaps the
second collective; the big MLP weight loads are emitted during the
attention phase.
"""

import sys

sys.path.insert(0, "/opt/trn_rl_repo")

from contextlib import ExitStack

import numpy as np
import ml_dtypes

import concourse.bacc as bacc
import concourse.mybir as mybir
import concourse.tile as tile
from concourse.bass_utils import run_bass_kernel_spmd

F32 = mybir.dt.float32
BF16 = mybir.dt.bfloat16
AF = mybir.ActivationFunctionType
ALU = mybir.AluOpType

B, P, D, H, DH = 2, 2048, 1024, 16, 64
T = B * P
FF = 4 * D
EPS = 1e-5
NCORES = 8
QL = T // NCORES
DC = D // 128
BIG = 30000.0
SCL = float(np.float64(D) ** -0.5)  # 1/32

NP_BF16 = ml_dtypes.bfloat16

# chunks whose LN1 stats run on DVE bn_stats (rest use ACT accumulators)
DVE_STATS = frozenset(i for i in range(32) if i % 8 < 3)


def build_nc():
    nc = bacc.Bacc(trn_type="TRN2", num_devices=NCORES)

    xall = nc.declare_dram_parameter("xall", [T, D], BF16, isOutput=False)
    xres = nc.declare_dram_parameter("xres", [QL, D], F32, isOutput=False)
    b2bc = nc.declare_dram_parameter("b2bc", [128, D], F32, isOutput=False)
    wq = nc.declare_dram_parameter("wq", [128, DC, 128], BF16, isOutput=False)
    wk = nc.declare_dram_parameter("wk", [128, DC, 128], BF16, isOutput=False)
    wv = nc.declare_dram_parameter("wv", [128, DC, 128], BF16, isOutput=False)
    bqk = nc.declare_dram_parameter("bqk", [128, 2], F32, isOutput=False)
    wp0 = nc.declare_dram_parameter("wp0", [64, 8, D], BF16, isOutput=False)
    wp1 = nc.declare_dram_parameter("wp1", [64, 8, D], BF16, isOutput=False)
    w1 = nc.declare_dram_parameter("w1", [128, DC, FF], BF16, isOutput=False)
    w2 = nc.declare_dram_parameter("w2", [128, 32, D], BF16, isOutput=False)
    b1p = nc.declare_dram_parameter("b1p", [128, 32], F32, isOutput=False)
    masks = nc.declare_dram_parameter("masks", [128, 512], BF16, isOutput=False)
    idm = nc.declare_dram_parameter("idm", [128, 128], BF16, isOutput=False)
    out = nc.declare_dram_parameter("out", [QL, D], F32, isOutput=True)

    cc_in = [nc.dram_tensor(f"cc_in{h}", [NCORES, 64, QL], BF16) for h in range(2)]
    cc_out = [nc.dram_tensor(f"cc_out{h}", [NCORES, 64, QL], BF16) for h in range(2)]
    RG = [list(range(NCORES))]

    with tile.TileContext(nc) as tc, ExitStack() as ctx:
        persist = ctx.enter_context(tc.tile_pool(name="persist", bufs=1))
        spool = ctx.enter_context(tc.tile_pool(name="spool", bufs=4))

        # ---- small persistent constants (emitted first; tiny DMAs)
        idm_sb = persist.tile([128, 128], BF16)
        nc.sync.dma_start(idm_sb[:], idm.ap())
        masks_sb = persist.tile([128, 512], BF16)
        nc.sync.dma_start(masks_sb[:], masks.ap())
        bqk_sb = persist.tile([128, 2], F32)
        nc.sync.dma_start(bqk_sb[:], bqk.ap())
        b1_sb = persist.tile([128, 32], F32)
        nc.sync.dma_start(b1_sb[:], b1p.ap())
        eps_t = persist.tile([128, 1], F32)
        nc.vector.memset(eps_t[:], EPS)

        # long-lived mid tensors + phase-E weights (outlive the attention scope)
        b2bc_sb = persist.tile([128, D], F32)
        xmpool = ctx.enter_context(tc.tile_pool(name="xmpool", bufs=1))
        xmid = xmpool.tile([128, 4, D], F32)
        wppool = ctx.enter_context(tc.tile_pool(name="wppool", bufs=1))
        wp0_sb = wppool.tile([64, 8, D], BF16)
        wp1_sb = wppool.tile([64, 8, D], BF16)
        xres_sb = wppool.tile([128, 4, D], F32)

        # attention-phase persistent activations
        apers_cm = tc.tile_pool(name="apers", bufs=1)
        apers = apers_cm.__enter__()
        qT = apers.tile([128, T], BF16)
        kT = apers.tile([128, T], BF16)
        v_sb = apers.tile([128, 32, 130], BF16)  # per head: 64 v + ones col
        oT = apers.tile([128, T], BF16)
        nc.vector.memset(v_sb[:, :, 64:65], 1.0)
        nc.vector.memset(v_sb[:, :, 129:130], 1.0)

        # ===== Phase B+C interleaved: LN1 -> h^T -> QKV per 512-token group
        with tc.tile_pool(name="wqkvp", bufs=1) as wqkvp, \
             tc.tile_pool(name="hTpool", bufs=1) as hTpool:
            wq_sb = wqkvp.tile([128, DC, 128], BF16)
            nc.sync.dma_start(wq_sb[:], wq.ap())
            wk_sb = wqkvp.tile([128, DC, 128], BF16)
            nc.sync.dma_start(wk_sb[:], wk.ap())
            wv_sb = wqkvp.tile([128, DC, 128], BF16)
            nc.sync.dma_start(wv_sb[:], wv.ap())
            hT = hTpool.tile([128, DC, T], BF16)

            with tc.tile_pool(name="xpool", bufs=4) as xpool, \
                 tc.tile_pool(name="hpool", bufs=4) as hpool, \
                 tc.tile_pool(name="qkps", bufs=2, space="PSUM") as qkps, \
                 tc.tile_pool(name="vps", bufs=2, space="PSUM") as vps:
                for grp in range(8):
                    for j in range(4):
                        tcnk = 4 * grp + j
                        xt = xpool.tile([128, D], BF16, tag="xt")
                        nc.sync.dma_start(
                            xt[:], xall.ap()[128 * tcnk:128 * (tcnk + 1), :])
                        mv0 = spool.tile([128, 1], F32, tag="mv0", name="mv0")
                        rstd = spool.tile([128, 1], F32, tag="rstd", name="rstd")
                        if tcnk in DVE_STATS:
                            st = spool.tile([128, 2, nc.vector.BN_STATS_DIM],
                                            F32, tag="st", name="st")
                            nc.vector.bn_stats(out=st[:, 0, :], in_=xt[:, 0:512])
                            nc.vector.bn_stats(out=st[:, 1, :], in_=xt[:, 512:1024])
                            mvv = spool.tile([128, 2], F32, tag="mvv", name="mvv")
                            nc.vector.bn_aggr(out=mvv[:], in_=st[:])
                            nc.vector.tensor_copy(mv0[:], mvv[:, 0:1])
                            nc.scalar.activation(out=rstd[:], in_=mvv[:, 1:2],
                                                 func=AF.Sqrt, bias=eps_t[:])
                        else:
                            jnk = spool.tile([128, D], BF16, tag="jnk", name="jnk")
                            sx = spool.tile([128, 1], F32, tag="sx", name="sx")
                            nc.scalar.activation(out=jnk[:], in_=xt[:],
                                                 func=AF.Copy, accum_out=sx[:])
                            jnk2 = spool.tile([128, D], BF16, tag="jnk2", name="jnk2")
                            sx2 = spool.tile([128, 1], F32, tag="sx2", name="sx2")
                            nc.scalar.activation(out=jnk2[:], in_=xt[:],
                                                 func=AF.Square, accum_out=sx2[:])
                            nc.vector.tensor_scalar_mul(mv0[:], sx[:], 1.0 / D)
                            # t = sx*mv0 - sx2 = -D*var ; rstd = sqrt(-t/D+eps)
                            tv = spool.tile([128, 1], F32, tag="tv", name="tv")
                            nc.vector.scalar_tensor_tensor(
                                out=tv[:], in0=sx[:], scalar=mv0[:], in1=sx2[:],
                                op0=ALU.mult, op1=ALU.subtract)
                            nc.scalar.activation(out=rstd[:], in_=tv[:],
                                                 func=AF.Sqrt, scale=-1.0 / D,
                                                 bias=eps_t[:])
                        nc.vector.reciprocal(out=rstd[:], in_=rstd[:])
                        ht = hpool.tile([128, D], BF16, tag="ht")
                        nc.vector.tensor_scalar(out=ht[:], in0=xt[:],
                                                scalar1=mv0[:], scalar2=rstd[:],
                                                op0=ALU.subtract, op1=ALU.mult)
                        nc.sync.dma_start_transpose(
                            hT[:, :, 128 * tcnk:128 * (tcnk + 1)], ht[:])

                    # QKV for this 512-token group
                    sl = slice(512 * grp, 512 * (grp + 1))
                    psq = qkps.tile([128, 512], F32, tag="psq")
                    for dc in range(DC):
                        nc.tensor.matmul(psq[:], wq_sb[:, dc, :], hT[:, dc, sl],
                                         start=(dc == 0), stop=(dc == DC - 1))
                    nc.vector.tensor_scalar_add(qT[:, sl], psq[:], bqk_sb[:, 0:1])
                    psk = qkps.tile([128, 512], F32, tag="psk")
                    for dc in range(DC):
                        nc.tensor.matmul(psk[:], wk_sb[:, dc, :], hT[:, dc, sl],
                                         start=(dc == 0), stop=(dc == DC - 1))
                    nc.vector.tensor_scalar_add(kT[:, sl], psk[:], bqk_sb[:, 1:2])
                    for j in range(4):
                        tcnk = 4 * grp + j
                        psv = vps.tile([128, 128], F32, tag="psv")
                        for dc in range(DC):
                            nc.tensor.matmul(
                                psv[:], hT[:, dc, 128 * tcnk:128 * (tcnk + 1)],
                                wv_sb[:, dc, :],
                                start=(dc == 0), stop=(dc == DC - 1))
                        vdst = v_sb[:, tcnk, :].rearrange(
                            "p (h c) -> p h c", h=2)[:, :, 0:64]
                        nc.vector.tensor_copy(
                            vdst, psv[:].rearrange("p (h c) -> p h c", h=2))

        # ===== Phase D: attention (phase-E inputs stream underneath)
        nc.sync.dma_start(wp0_sb[:], wp0.ap())
        nc.sync.dma_start(wp1_sb[:], wp1.ap())
        nc.sync.dma_start(b2bc_sb[:], b2bc.ap())
        nc.sync.dma_start(xres_sb[:], xres.ap().rearrange("(r p) d -> p r d", p=128))
        with tc.tile_pool(name="apool", bufs=2) as apool, \
             tc.tile_pool(name="rpool", bufs=3) as rpool, \
             tc.tile_pool(name="spsum", bufs=2, space="PSUM") as spsum, \
             tc.tile_pool(name="opsum", bufs=2, space="PSUM") as opsum:
            for hh in range(2):
                hp = slice(64 * hh, 64 * (hh + 1))
                vh = slice(65 * hh, 65 * (hh + 1))
                for b in range(B):
                    for qb in range(8):
                        qsl = slice(2048 * b + 256 * qb, 2048 * b + 256 * (qb + 1))
                        nch = 2 * qb + 2
                        att = apool.tile([128, 16, 256], BF16, tag="att")
                        for g0 in range(0, nch, 4):
                            gw = min(4, nch - g0)
                            sps = spsum.tile([128, 4, 256], F32, tag="sps")
                            for kc in range(g0, g0 + gw):
                                ksl = slice(2048 * b + 128 * kc,
                                            2048 * b + 128 * (kc + 1))
                                diag = kc >= nch - 2
                                if diag:
                                    # pre-zero + causal mask for this diag chunk
                                    msl = slice(256 * (kc - nch + 2),
                                                256 * (kc - nch + 3))
                                    nc.tensor.matmul(sps[:, kc - g0, :],
                                                     idm_sb[:], masks_sb[:, msl],
                                                     start=True, stop=False,
                                                     skip_group_check=True)
                                nc.tensor.matmul(sps[:, kc - g0, :],
                                                 kT[hp, ksl], qT[hp, qsl],
                                                 start=(not diag), stop=True,
                                                 skip_group_check=True)
                            nc.scalar.activation(out=att[:, g0:g0 + gw, :],
                                                 in_=sps[:, 0:gw, :],
                                                 func=AF.Exp, scale=SCL)
                        ops = opsum.tile([65, 256], F32, tag="ops")
                        for kc in range(nch):
                            nc.tensor.matmul(ops[:], v_sb[:, 16 * b + kc, vh],
                                             att[:, kc, :],
                                             start=(kc == 0), stop=(kc == nch - 1))
                        rec = rpool.tile([1, 256], F32, tag="rec")
                        nc.vector.reciprocal(out=rec[:], in_=ops[64:65, :])
                        rb = rpool.tile([64, 256], F32, tag="rb")
                        nc.gpsimd.partition_broadcast(rb[:], rec[0:1, :])
                        nc.vector.tensor_tensor(out=oT[hp, qsl], in0=ops[0:64, :],
                                                in1=rb[:], op=ALU.mult)
                nc.sync.dma_start(
                    cc_in[hh].ap().rearrange("j p t -> p j t"),
                    oT[hp, :].rearrange("p (j t) -> p j t", j=NCORES))
                nc.gpsimd.collective_compute(
                    "AllToAll", ALU.bypass, replica_groups=RG,
                    ins=[cc_in[hh].ap()], outs=[cc_out[hh].ap()])

        apers_cm.__exit__(None, None, None)

        # ===== Phase E: Wp (split halves to overlap collective 2) + residual
        with tc.tile_pool(name="epool", bufs=1) as epool:
            oT_a = epool.tile([64, 8, QL], BF16)
            nc.sync.dma_start(oT_a[:], cc_out[0].ap().rearrange("s p t -> p s t"))
            xmid0 = epool.tile([128, 4, D], F32)
            with tc.tile_pool(name="xps", bufs=2, space="PSUM") as xps:
                for r in range(4):
                    for dh in range(2):
                        dsl = slice(512 * dh, 512 * (dh + 1))
                        psx = xps.tile([128, 512], F32, tag="psx")
                        for s in range(8):
                            nc.tensor.matmul(
                                psx[:], oT_a[:, s, 128 * r:128 * (r + 1)],
                                wp0_sb[:, s, dsl],
                                start=(s == 0), stop=(s == 7))
                        nc.scalar.activation(out=xmid0[:, r, dsl], in_=psx[:],
                                             func=AF.Copy)
                oT_b = epool.tile([64, 8, QL], BF16)
                nc.sync.dma_start(oT_b[:],
                                  cc_out[1].ap().rearrange("s p t -> p s t"))
                for r in range(4):
                    for dh in range(2):
                        dsl = slice(512 * dh, 512 * (dh + 1))
                        psx = xps.tile([128, 512], F32, tag="psx")
                        for s in range(8):
                            nc.tensor.matmul(
                                psx[:], oT_b[:, s, 128 * r:128 * (r + 1)],
                                wp1_sb[:, s, dsl],
                                start=(s == 0), stop=(s == 7))
                        nc.vector.tensor_tensor(out=xmid[:, r, dsl], in0=psx[:],
                                                in1=xmid0[:, r, dsl], op=ALU.add)
                for r in range(4):
                    nc.vector.tensor_tensor(out=xmid[:, r, :], in0=xmid[:, r, :],
                                            in1=xres_sb[:, r, :], op=ALU.add)

            # ===== Phase F: LN2 -> h2^T (bf16, PE transpose)
            with tc.tile_pool(name="h2pool", bufs=1) as h2pool:
                h2T = h2pool.tile([128, DC, QL], BF16)
                with tc.tile_pool(name="hp2", bufs=2) as hp2, \
                     tc.tile_pool(name="trps", bufs=2, space="PSUM") as trps:
                    for r in range(4):
                        st = spool.tile([128, 2, nc.vector.BN_STATS_DIM],
                                        F32, tag="st2", name="st2")
                        nc.vector.bn_stats(out=st[:, 0, :], in_=xmid[:, r, 0:512])
                        nc.vector.bn_stats(out=st[:, 1, :], in_=xmid[:, r, 512:1024])
                        mvv = spool.tile([128, 2], F32, tag="mvv2", name="mvv2")
                        nc.vector.bn_aggr(out=mvv[:], in_=st[:])
                        rstd = spool.tile([128, 1], F32, tag="rs2", name="rs2")
                        nc.scalar.activation(out=rstd[:], in_=mvv[:, 1:2],
                                             func=AF.Sqrt, bias=eps_t[:])
                        nc.vector.reciprocal(out=rstd[:], in_=rstd[:])
                        h2 = hp2.tile([128, D], BF16, tag="h2")
                        nc.vector.tensor_scalar(out=h2[:], in0=xmid[:, r, :],
                                                scalar1=mvv[:, 0:1], scalar2=rstd[:],
                                                op0=ALU.subtract, op1=ALU.mult)
                        for g4 in range(2):
                            tp = trps.tile([128, 4, 128], BF16, tag="tp")
                            for j in range(4):
                                dc = 4 * g4 + j
                                nc.tensor.transpose(
                                    tp[:, j, :], h2[:, 128 * dc:128 * (dc + 1)],
                                    idm_sb[:])
                            nc.vector.tensor_copy(
                                h2T[:, 4 * g4:4 * g4 + 4, 128 * r:128 * (r + 1)],
                                tp[:])
                for r in range(4):
                    nc.vector.tensor_tensor(out=xmid[:, r, :], in0=xmid[:, r, :],
                                            in1=b2bc_sb[:], op=ALU.add)

                # ===== Phase G: MLP
                with tc.tile_pool(name="mpool", bufs=1) as mpool, \
                     tc.tile_pool(name="w1p", bufs=3) as w1p, \
                     tc.tile_pool(name="ost", bufs=1) as ostp:
                    mT = mpool.tile([128, 32, QL], BF16)
                    with tc.tile_pool(name="mps", bufs=2, space="PSUM") as mps:
                        for fc in range(32):
                            w1c = w1p.tile([128, DC, 128], BF16, tag="w1c")
                            nc.sync.dma_start(w1c[:],
                                              w1.ap()[:, :, 128 * fc:128 * (fc + 1)])
                            psm = mps.tile([128, 512], F32, tag="psm")
                            for dc in range(DC):
                                nc.tensor.matmul(psm[:], w1c[:, dc, :],
                                                 h2T[:, dc, :],
                                                 start=(dc == 0), stop=(dc == DC - 1))
                            nc.scalar.activation(out=mT[:, fc, :], in_=psm[:],
                                                 func=AF.Gelu, bias=b1_sb[:, fc:fc + 1])
                    ost = ostp.tile([128, 4, D], F32)
                    ops2 = ctx.enter_context(
                        tc.tile_pool(name="ops2", bufs=1, space="PSUM"))
                    psos = [ops2.tile([128, 512], F32, tag=f"pso{u}",
                                      name=f"pso{u}") for u in range(8)]
                    for fc in range(32):
                        w2c = w1p.tile([128, D], BF16, tag="w2c")
                        nc.sync.dma_start(w2c[:], w2.ap()[:, fc, :])
                        for u in range(8):
                            r, dh = u // 2, u % 2
                            nc.tensor.matmul(
                                psos[u][:],
                                mT[:, fc, 128 * r:128 * (r + 1)],
                                w2c[:, 512 * dh:512 * (dh + 1)],
                                start=(fc == 0), stop=(fc == 31))
                    for u in range(8):
                        r, dh = u // 2, u % 2
                        dsl = slice(512 * dh, 512 * (dh + 1))
                        nc.vector.tensor_tensor(out=ost[:, r, dsl], in0=psos[u][:],
                                                in1=xmid[:, r, dsl], op=ALU.add)
                    nc.sync.dma_start(out.ap().rearrange("(r p) d -> p r d", p=128),
                                      ost[:])

    nc.compile()
    return nc


_NC_CACHE = {}


def _get_nc():
    if "nc" not in _NC_CACHE:
        _NC_CACHE["nc"] = build_nc()
    return _NC_CACHE["nc"]


def _host_pack(inputs):
    f32 = np.float32
    x = np.asarray(inputs["x"], f32).reshape(T, D)
    Wq = np.asarray(inputs["Wq"], f32)
    Wk = np.asarray(inputs["Wk"], f32)
    Wv = np.asarray(inputs["Wv"], f32)
    Wp = np.asarray(inputs["Wp"], f32)
    bp = np.asarray(inputs["bp"], f32)
    W1 = np.asarray(inputs["W1"], f32)
    b1 = np.asarray(inputs["b1"], f32)
    W2 = np.asarray(inputs["W2"], f32)
    b2 = np.asarray(inputs["b2"], f32)
    g1 = np.asarray(inputs["g1"], f32)
    be1 = np.asarray(inputs["be1"], f32)
    g2 = np.asarray(inputs["g2"], f32)
    be2 = np.asarray(inputs["be2"], f32)

    Wq_f = Wq * g1[None, :, None]
    Wk_f = Wk * g1[None, :, None]
    Wv_f = Wv * g1[None, :, None]
    bq_f = np.einsum("d,hde->he", be1, Wq)
    bk_f = np.einsum("d,hde->he", be1, Wk)
    bv_f = np.einsum("d,hde->he", be1, Wv)
    res_const = bp + bv_f.reshape(H * DH) @ Wp

    W1_f = W1 * g2[:, None]
    b1_f = (b1 + be2 @ W1).astype(f32)

    xall = x.astype(NP_BF16)
    b2bc = np.ascontiguousarray(np.broadcast_to(b2, (128, D))).astype(f32)
    w1_d = np.ascontiguousarray(
        W1_f.reshape(DC, 128, FF).transpose(1, 0, 2).astype(NP_BF16))
    w2_d = np.ascontiguousarray(
        W2.reshape(32, 128, D).transpose(1, 0, 2).astype(NP_BF16))
    wp_r = Wp.reshape(8, 2, 64, D)  # [s, half, 64, D]
    wp0_d = np.ascontiguousarray(wp_r[:, 0].transpose(1, 0, 2).astype(NP_BF16))
    wp1_d = np.ascontiguousarray(wp_r[:, 1].transpose(1, 0, 2).astype(NP_BF16))
    b1p = np.ascontiguousarray(b1_f.reshape(32, 128).T)

    masks = np.zeros((128, 512), f32)
    ii = np.arange(128)[:, None]
    jj = np.arange(256)[None, :]
    masks[:, 0:256] = np.where(ii <= jj, 0.0, -BIG)
    masks[:, 256:512] = np.where(128 + ii <= jj, 0.0, -BIG)
    masks = masks.astype(NP_BF16)
    idm = np.eye(128, dtype=f32).astype(NP_BF16)

    shared = dict(
        xall=xall.view(np.uint16), b2bc=b2bc,
        w1=w1_d.view(np.uint16), w2=w2_d.view(np.uint16),
        wp0=wp0_d.view(np.uint16), wp1=wp1_d.view(np.uint16), b1p=b1p,
        masks=masks.view(np.uint16), idm=idm.view(np.uint16),
    )

    in_maps = []
    for c in range(NCORES):
        h0, h1 = 2 * c, 2 * c + 1
        wq_c = np.concatenate([Wq_f[h0], Wq_f[h1]], axis=1)
        wk_c = np.concatenate([Wk_f[h0], Wk_f[h1]], axis=1)
        wv_c = np.concatenate([Wv_f[h0], Wv_f[h1]], axis=1)
        pack = lambda w: np.ascontiguousarray(
            w.reshape(DC, 128, 128).transpose(1, 0, 2).astype(NP_BF16)).view(np.uint16)
        bqk_c = np.stack(
            [np.concatenate([bq_f[h0], bq_f[h1]]),
             np.concatenate([bk_f[h0], bk_f[h1]])], axis=1).astype(f32)
        xres_c = (x[QL * c:QL * (c + 1)] + res_const).astype(f32)
        in_maps.append(dict(
            shared, wq=pack(wq_c), wk=pack(wk_c), wv=pack(wv_c),
            bqk=np.ascontiguousarray(bqk_c), xres=np.ascontiguousarray(xres_c)))
    return in_maps


def _unshard(results):
    out = np.empty((T, D), np.float32)
    for c in range(NCORES):
        out[QL * c:QL * (c + 1)] = results[c]["out"]
    return out.reshape(B, P, D)


def kernel(**inputs):
    in_maps = _host_pack(inputs)
    nc = _get_nc()
    res = run_bass_kernel_spmd(nc, in_maps, core_ids=list(range(NCORES)))
    return _unshard(res.results)


# revision 12
# speedup vs baseline: 1.5684x; 1.0294x over previous
"""Trainium2 Bass kernel for a dense transformer block.

reference: x -> LN1 -> 16-head causal attention (+residual) -> LN2 -> MLP
(+residual), x: [2, 2048, 1024] fp32.

Sharding (v3): tensor-parallel attention + sequence-parallel MLP.
Core c (of 8) computes heads {2c, 2c+1} over ALL 4096 tokens (both
batches): LN1 is replicated, QKV projection / scores / softmax / AV run
on the core's 2 heads only.  Two bf16 8-core AllToAlls (one per local
head) reshard the attention output o^T from head-major to
sequence-major; core c then owns tokens [512c, 512c+512) for the output
projection, LN2 and the MLP, and writes exactly those 512 rows.

All matmuls run in bf16 (1 PE cycle/row) with fp32 psum accumulation;
residuals are fp32.  fp8 was measured (numpy-emulated against the jax
oracle) to blow most of the 2e-2 error budget at any single site, so it
is not used.

Scheduling tricks: h^T comes from the XBAR DMA-transpose engine (no PE
transposes, no psum evacuations); LN1 emission is interleaved with the
QKV matmuls per 512-token group so the PE never starves; LN1 stats are
split between ACT (Copy/Square with free-dim accumulators) and DVE
(bn_stats) to balance engines; causal masks for the two diagonal
kv-chunks are injected into the scores psum by an identity-stationary
matmul on the PE; softmax denominators ride the AV matmul as an
all-ones column of V; the normalizer broadcast runs on gpsimd; exp
applies the D^-0.5 scale via the ACT scale field; the output projection
is split into two half-contractions so the first half overlaps the
second collective; the big MLP weight loads are emitted during the
attention phase.
"""

import sys

sys.path.insert(0, "/opt/trn_rl_repo")

from contextlib import ExitStack

import numpy as np
import ml_dtypes

import concourse.bacc as bacc
import concourse.mybir as mybir
import concourse.tile as tile
from concourse.bass_utils import run_bass_kernel_spmd

F32 = mybir.dt.float32
BF16 = mybir.dt.bfloat16
AF = mybir.ActivationFunctionType
ALU = mybir.AluOpType

B, P, D, H, DH = 2, 2048, 1024, 16, 64
T = B * P
FF = 4 * D
EPS = 1e-5
NCORES = 8
QL = T // NCORES
DC = D // 128
BIG = 30000.0
SCL = float(np.float64(D) ** -0.5)  # 1/32

NP_BF16 = ml_dtypes.bfloat16

# chunks whose LN1 stats run on DVE bn_stats (rest use ACT accumulators)
DVE_STATS = frozenset(i for i in range(32) if i % 8 < 3)


def build_nc():
    nc = bacc.Bacc(trn_type="TRN2", num_devices=NCORES)

    xall = nc.declare_dram_parameter("xall", [T, D], BF16, isOutput=False)
    xres = nc.declare_dram_parameter("xres", [QL, D], F32, isOutput=False)
    b2bc = nc.declare_dram_parameter("b2bc", [128, D], F32, isOutput=False)
    wq = nc.declare_dram_parameter("wq", [128, DC, 128], BF16, isOutput=False)
    wk = nc.declare_dram_parameter("wk", [128, DC, 128], BF16, isOutput=False)
    wv = nc.declare_dram_parameter("wv", [128, DC, 128], BF16, isOutput=False)
    bqk = nc.declare_dram_parameter("bqk", [128, 2], F32, isOutput=False)
    wp0 = nc.declare_dram_parameter("wp0", [64, 8, D], BF16, isOutput=False)
    wp1 = nc.declare_dram_parameter("wp1", [64, 8, D], BF16, isOutput=False)
    w1 = nc.declare_dram_parameter("w1", [128, DC, FF], BF16, isOutput=False)
    w2 = nc.declare_dram_parameter("w2", [128, 32, D], BF16, isOutput=False)
    b1p = nc.declare_dram_parameter("b1p", [128, 32], F32, isOutput=False)
    masks = nc.declare_dram_parameter("masks", [128, 512], BF16, isOutput=False)
    idm = nc.declare_dram_parameter("idm", [128, 128], BF16, isOutput=False)
    out = nc.declare_dram_parameter("out", [QL, D], F32, isOutput=True)

    cc_in = [nc.dram_tensor(f"cc_in{h}", [NCORES, 64, QL], BF16) for h in range(2)]
    cc_out = [nc.dram_tensor(f"cc_out{h}", [NCORES, 64, QL], BF16) for h in range(2)]
    RG = [list(range(NCORES))]

    with tile.TileContext(nc) as tc, ExitStack() as ctx:
        persist = ctx.enter_context(tc.tile_pool(name="persist", bufs=1))
        spool = ctx.enter_context(tc.tile_pool(name="spool", bufs=4))

        # ---- small persistent constants (emitted first; tiny DMAs)
        idm_sb = persist.tile([128, 128], BF16)
        nc.sync.dma_start(idm_sb[:], idm.ap())
        masks_sb = persist.tile([128, 512], BF16)
        nc.sync.dma_start(masks_sb[:], masks.ap())
        bqk_sb = persist.tile([128, 2], F32)
        nc.sync.dma_start(bqk_sb[:], bqk.ap())
        b1_sb = persist.tile([128, 32], F32)
        nc.sync.dma_start(b1_sb[:], b1p.ap())
        eps_t = persist.tile([128, 1], F32)
        nc.vector.memset(eps_t[:], EPS)

        # long-lived mid tensors + phase-E weights (outlive the attention scope)
        b2bc_sb = persist.tile([128, D], F32)
        xmpool = ctx.enter_context(tc.tile_pool(name="xmpool", bufs=1))
        xmid = xmpool.tile([128, 4, D], F32)
        wppool = ctx.enter_context(tc.tile_pool(name="wppool", bufs=1))
        wp0_sb = wppool.tile([64, 8, D], BF16)
        wp1_sb = wppool.tile([64, 8, D], BF16)
        xres_sb = wppool.tile([128, 4, D], F32)

        # attention-phase persistent activations
        apers_cm = tc.tile_pool(name="apers", bufs=1)
        apers = apers_cm.__enter__()
        qT = apers.tile([128, T], BF16)
        kT = apers.tile([128, T], BF16)
        v_sb = apers.tile([128, 32, 130], BF16)  # per head: 64 v + ones col
        oT = apers.tile([128, T], BF16)
        nc.vector.memset(v_sb[:, :, 64:65], 1.0)
        nc.vector.memset(v_sb[:, :, 129:130], 1.0)

        # ===== Phase B+C interleaved: LN1 -> h^T -> QKV per 512-token group
        with tc.tile_pool(name="wqkvp", bufs=1) as wqkvp, \
             tc.tile_pool(name="hTpool", bufs=1) as hTpool:
            wq_sb = wqkvp.tile([128, DC, 128], BF16)
            nc.sync.dma_start(wq_sb[:], wq.ap())
            wk_sb = wqkvp.tile([128, DC, 128], BF16)
            nc.sync.dma_start(wk_sb[:], wk.ap())
            wv_sb = wqkvp.tile([128, DC, 128], BF16)
            nc.sync.dma_start(wv_sb[:], wv.ap())
            hT = hTpool.tile([128, DC, T], BF16)

            with tc.tile_pool(name="xpool", bufs=2) as xpool, \
                 tc.tile_pool(name="hpool", bufs=2) as hpool, \
                 tc.tile_pool(name="qkps", bufs=2, space="PSUM") as qkps, \
                 tc.tile_pool(name="vps", bufs=2, space="PSUM") as vps:
                jnkp = hpool  # junk reuse: ACT is in-order, WAR is harmless
                for grp in range(8):
                    xts, mv0s, rstds = [], [], []
                    for j in range(4):
                        tcnk = 4 * grp + j
                        xt = xpool.tile([128, D], BF16, tag=f"xt{j}",
                                        name=f"xt{j}")
                        nc.sync.dma_start(
                            xt[:], xall.ap()[128 * tcnk:128 * (tcnk + 1), :])
                        xts.append(xt)
                        mv0s.append(spool.tile([128, 1], F32, tag=f"mv0{j}",
                                               name=f"mv0{j}"))
                        rstds.append(spool.tile([128, 1], F32, tag=f"rstd{j}",
                                                name=f"rstd{j}"))
                    # stats: chunk 0 via DVE bn_stats, chunks 1-3 via ACT accum
                    sxs = {}
                    for j in range(1, 4):
                        jnk = jnkp.tile([128, D], BF16, tag="jnk", name="jnk")
                        sx = spool.tile([128, 1], F32, tag=f"sx{j}", name=f"sx{j}")
                        nc.scalar.activation(out=jnk[:], in_=xts[j][:],
                                             func=AF.Copy, accum_out=sx[:])
                        jnk2 = jnkp.tile([128, D], BF16, tag="jnk2", name="jnk2")
                        sx2 = spool.tile([128, 1], F32, tag=f"sx2{j}",
                                         name=f"sx2{j}")
                        nc.scalar.activation(out=jnk2[:], in_=xts[j][:],
                                             func=AF.Square, accum_out=sx2[:])
                        sxs[j] = (sx, sx2)
                    st = spool.tile([128, 2, nc.vector.BN_STATS_DIM],
                                    F32, tag="st", name="st")
                    nc.vector.bn_stats(out=st[:, 0, :], in_=xts[0][:, 0:512])
                    nc.vector.bn_stats(out=st[:, 1, :], in_=xts[0][:, 512:1024])
                    mvv = spool.tile([128, 2], F32, tag="mvv", name="mvv")
                    nc.vector.bn_aggr(out=mvv[:], in_=st[:])
                    nc.vector.tensor_copy(mv0s[0][:], mvv[:, 0:1])
                    tvs = {}
                    for j in range(1, 4):
                        sx, sx2 = sxs[j]
                        nc.vector.tensor_scalar_mul(mv0s[j][:], sx[:], 1.0 / D)
                        tv = spool.tile([128, 1], F32, tag=f"tv{j}", name=f"tv{j}")
                        nc.vector.scalar_tensor_tensor(
                            out=tv[:], in0=sx[:], scalar=mv0s[j][:], in1=sx2[:],
                            op0=ALU.mult, op1=ALU.subtract)
                        tvs[j] = tv
                    nc.scalar.activation(out=rstds[0][:], in_=mvv[:, 1:2],
                                         func=AF.Sqrt, bias=eps_t[:])
                    for j in range(1, 4):
                        nc.scalar.activation(out=rstds[j][:], in_=tvs[j][:],
                                             func=AF.Sqrt, scale=-1.0 / D,
                                             bias=eps_t[:])
                    for j in range(4):
                        nc.vector.reciprocal(out=rstds[j][:], in_=rstds[j][:])
                    for j in range(4):
                        tcnk = 4 * grp + j
                        ht = hpool.tile([128, D], BF16, tag="ht")
                        nc.vector.tensor_scalar(out=ht[:], in0=xts[j][:],
                                                scalar1=mv0s[j][:],
                                                scalar2=rstds[j][:],
                                                op0=ALU.subtract, op1=ALU.mult)
                        nc.sync.dma_start_transpose(
                            hT[:, :, 128 * tcnk:128 * (tcnk + 1)], ht[:])

                    # QKV for this 512-token group
                    sl = slice(512 * grp, 512 * (grp + 1))
                    psq = qkps.tile([128, 512], F32, tag="psq")
                    for dc in range(DC):
                        nc.tensor.matmul(psq[:], wq_sb[:, dc, :], hT[:, dc, sl],
                                         start=(dc == 0), stop=(dc == DC - 1))
                    nc.vector.tensor_scalar_add(qT[:, sl], psq[:], bqk_sb[:, 0:1])
                    psk = qkps.tile([128, 512], F32, tag="psk")
                    for dc in range(DC):
                        nc.tensor.matmul(psk[:], wk_sb[:, dc, :], hT[:, dc, sl],
                                         start=(dc == 0), stop=(dc == DC - 1))
                    nc.vector.tensor_scalar_add(kT[:, sl], psk[:], bqk_sb[:, 1:2])
                    for j in range(4):
                        tcnk = 4 * grp + j
                        psv = vps.tile([128, 128], F32, tag="psv")
                        for dc in range(DC):
                            nc.tensor.matmul(
                                psv[:], hT[:, dc, 128 * tcnk:128 * (tcnk + 1)],
                                wv_sb[:, dc, :],
                                start=(dc == 0), stop=(dc == DC - 1))
                        vdst = v_sb[:, tcnk, :].rearrange(
                            "p (h c) -> p h c", h=2)[:, :, 0:64]
                        nc.vector.tensor_copy(
                            vdst, psv[:].rearrange("p (h c) -> p h c", h=2))

        # ===== Phase D: attention (phase-E inputs stream underneath)
        nc.sync.dma_start(wp0_sb[:], wp0.ap())
        nc.sync.dma_start(wp1_sb[:], wp1.ap())
        nc.sync.dma_start(b2bc_sb[:], b2bc.ap())
        nc.sync.dma_start(xres_sb[:], xres.ap().rearrange("(r p) d -> p r d", p=128))
        with tc.tile_pool(name="apool", bufs=2) as apool, \
             tc.tile_pool(name="rpool", bufs=3) as rpool, \
             tc.tile_pool(name="spsum", bufs=3, space="PSUM") as spsum, \
             tc.tile_pool(name="opsum", bufs=2, space="PSUM") as opsum:
            for hh in range(2):
                hp = slice(64 * hh, 64 * (hh + 1))
                vh = slice(65 * hh, 65 * (hh + 1))
                for b in range(B):
                    for qb in range(8):
                        qsl = slice(2048 * b + 256 * qb, 2048 * b + 256 * (qb + 1))
                        nch = 2 * qb + 2
                        att = apool.tile([128, 16, 256], BF16, tag="att")
                        for g0 in range(0, nch, 4):
                            gw = min(4, nch - g0)
                            sps = spsum.tile([128, 4, 256], F32, tag="sps")
                            for kc in range(g0, g0 + gw):
                                ksl = slice(2048 * b + 128 * kc,
                                            2048 * b + 128 * (kc + 1))
                                diag = kc >= nch - 2
                                if diag:
                                    # pre-zero + causal mask for this diag chunk
                                    msl = slice(256 * (kc - nch + 2),
                                                256 * (kc - nch + 3))
                                    nc.tensor.matmul(sps[:, kc - g0, :],
                                                     idm_sb[:], masks_sb[:, msl],
                                                     start=True, stop=False,
                                                     skip_group_check=True)
                                nc.tensor.matmul(sps[:, kc - g0, :],
                                                 kT[hp, ksl], qT[hp, qsl],
                                                 start=(not diag), stop=True,
                                                 skip_group_check=True)
                            nc.scalar.activation(out=att[:, g0:g0 + gw, :],
                                                 in_=sps[:, 0:gw, :],
                                                 func=AF.Exp, scale=SCL)
                        ops = opsum.tile([65, 256], F32, tag="ops")
                        for kc in range(nch):
                            nc.tensor.matmul(ops[:], v_sb[:, 16 * b + kc, vh],
                                             att[:, kc, :],
                                             start=(kc == 0), stop=(kc == nch - 1))
                        rec = rpool.tile([1, 256], F32, tag="rec")
                        nc.vector.reciprocal(out=rec[:], in_=ops[64:65, :])
                        rb = rpool.tile([64, 256], F32, tag="rb")
                        nc.gpsimd.partition_broadcast(rb[:], rec[0:1, :])
                        nc.vector.tensor_tensor(out=oT[hp, qsl], in0=ops[0:64, :],
                                                in1=rb[:], op=ALU.mult)
                nc.sync.dma_start(
                    cc_in[hh].ap().rearrange("j p t -> p j t"),
                    oT[hp, :].rearrange("p (j t) -> p j t", j=NCORES))
                nc.gpsimd.collective_compute(
                    "AllToAll", ALU.bypass, replica_groups=RG,
                    ins=[cc_in[hh].ap()], outs=[cc_out[hh].ap()])

        apers_cm.__exit__(None, None, None)

        # ===== Phase E: Wp (split halves to overlap collective 2) + residual
        with tc.tile_pool(name="epool", bufs=1) as epool:
            oT_a = epool.tile([64, 8, QL], BF16)
            nc.sync.dma_start(oT_a[:], cc_out[0].ap().rearrange("s p t -> p s t"))
            xmid0 = epool.tile([128, 4, D], F32)
            with tc.tile_pool(name="xps", bufs=2, space="PSUM") as xps:
                for r in range(4):
                    for dh in range(2):
                        dsl = slice(512 * dh, 512 * (dh + 1))
                        psx = xps.tile([128, 512], F32, tag="psx")
                        for s in range(8):
                            nc.tensor.matmul(
                                psx[:], oT_a[:, s, 128 * r:128 * (r + 1)],
                                wp0_sb[:, s, dsl],
                                start=(s == 0), stop=(s == 7))
                        nc.scalar.activation(out=xmid0[:, r, dsl], in_=psx[:],
                                             func=AF.Copy)
                oT_b = epool.tile([64, 8, QL], BF16)
                nc.sync.dma_start(oT_b[:],
                                  cc_out[1].ap().rearrange("s p t -> p s t"))
                for r in range(4):
                    for dh in range(2):
                        dsl = slice(512 * dh, 512 * (dh + 1))
                        psx = xps.tile([128, 512], F32, tag="psx")
                        for s in range(8):
                            nc.tensor.matmul(
                                psx[:], oT_b[:, s, 128 * r:128 * (r + 1)],
                                wp1_sb[:, s, dsl],
                                start=(s == 0), stop=(s == 7))
                        nc.vector.tensor_tensor(out=xmid[:, r, dsl], in0=psx[:],
                                                in1=xmid0[:, r, dsl], op=ALU.add)
                for r in range(4):
                    nc.vector.tensor_tensor(out=xmid[:, r, :], in0=xmid[:, r, :],
                                            in1=xres_sb[:, r, :], op=ALU.add)

            # ===== Phase F: LN2 -> h2^T (bf16, PE transpose)
            with tc.tile_pool(name="h2pool", bufs=1) as h2pool:
                h2T = h2pool.tile([128, DC, QL], BF16)
                with tc.tile_pool(name="hp2", bufs=2) as hp2, \
                     tc.tile_pool(name="trps", bufs=2, space="PSUM") as trps:
                    for r in range(4):
                        st = spool.tile([128, 2, nc.vector.BN_STATS_DIM],
                                        F32, tag="st2", name="st2")
                        nc.vector.bn_stats(out=st[:, 0, :], in_=xmid[:, r, 0:512])
                        nc.vector.bn_stats(out=st[:, 1, :], in_=xmid[:, r, 512:1024])
                        mvv = spool.tile([128, 2], F32, tag="mvv2", name="mvv2")
                        nc.vector.bn_aggr(out=mvv[:], in_=st[:])
                        rstd = spool.tile([128, 1], F32, tag="rs2", name="rs2")
                        nc.scalar.activation(out=rstd[:], in_=mvv[:, 1:2],
                                             func=AF.Sqrt, bias=eps_t[:])
                        nc.vector.reciprocal(out=rstd[:], in_=rstd[:])
                        h2 = hp2.tile([128, D], BF16, tag="h2")
                        nc.vector.tensor_scalar(out=h2[:], in0=xmid[:, r, :],
                                                scalar1=mvv[:, 0:1], scalar2=rstd[:],
                                                op0=ALU.subtract, op1=ALU.mult)
                        for g4 in range(2):
                            tp = trps.tile([128, 4, 128], BF16, tag="tp")
                            for j in range(4):
                                dc = 4 * g4 + j
                                nc.tensor.transpose(
                                    tp[:, j, :], h2[:, 128 * dc:128 * (dc + 1)],
                                    idm_sb[:])
                            nc.vector.tensor_copy(
                                h2T[:, 4 * g4:4 * g4 + 4, 128 * r:128 * (r + 1)],
                                tp[:])
                for r in range(4):
                    nc.vector.tensor_tensor(out=xmid[:, r, :], in0=xmid[:, r, :],
                                            in1=b2bc_sb[:], op=ALU.add)

                # ===== Phase G: MLP
                with tc.tile_pool(name="mpool", bufs=1) as mpool, \
                     tc.tile_pool(name="w1p", bufs=3) as w1p, \
                     tc.tile_pool(name="ost", bufs=1) as ostp:
                    mT = mpool.tile([128, 32, QL], BF16)
                    with tc.tile_pool(name="mps", bufs=2, space="PSUM") as mps:
                        for fq in range(8):
                            w1c = w1p.tile([128, DC, 512], BF16, tag="w1c")
                            nc.sync.dma_start(
                                w1c[:], w1.ap()[:, :, 512 * fq:512 * (fq + 1)])
                            for fj in range(4):
                                fc = 4 * fq + fj
                                psm = mps.tile([128, 512], F32, tag="psm")
                                for dc in range(DC):
                                    nc.tensor.matmul(
                                        psm[:],
                                        w1c[:, dc, 128 * fj:128 * (fj + 1)],
                                        h2T[:, dc, :],
                                        start=(dc == 0), stop=(dc == DC - 1))
                                nc.scalar.activation(out=mT[:, fc, :], in_=psm[:],
                                                     func=AF.Gelu,
                                                     bias=b1_sb[:, fc:fc + 1])
                    ost = ostp.tile([128, 4, D], F32)
                    ops2 = ctx.enter_context(
                        tc.tile_pool(name="ops2", bufs=1, space="PSUM"))
                    psos = [ops2.tile([128, 512], F32, tag=f"pso{u}",
                                      name=f"pso{u}") for u in range(4)]
                    for rg in range(2):      # rows 0-1, then rows 2-3
                        for fc in range(32):
                            w2c = w1p.tile([128, D], BF16, tag="w2c")
                            nc.sync.dma_start(w2c[:], w2.ap()[:, fc, :])
                            for u in range(4):
                                r, dh = 2 * rg + u // 2, u % 2
                                nc.tensor.matmul(
                                    psos[u][:],
                                    mT[:, fc, 128 * r:128 * (r + 1)],
                                    w2c[:, 512 * dh:512 * (dh + 1)],
                                    start=(fc == 0), stop=(fc == 31))
                        for u in range(4):
                            r, dh = 2 * rg + u // 2, u % 2
                            dsl = slice(512 * dh, 512 * (dh + 1))
                            nc.vector.tensor_tensor(
                                out=ost[:, r, dsl], in0=psos[u][:],
                                in1=xmid[:, r, dsl], op=ALU.add)
                        for rr in (2 * rg, 2 * rg + 1):
                            nc.sync.dma_start(
                                out.ap()[128 * rr:128 * (rr + 1), :],
                                ost[:, rr, :])

    nc.compile()
    return nc


_NC_CACHE = {}


def _get_nc():
    if "nc" not in _NC_CACHE:
        _NC_CACHE["nc"] = build_nc()
    return _NC_CACHE["nc"]


def _host_pack(inputs):
    f32 = np.float32
    x = np.asarray(inputs["x"], f32).reshape(T, D)
    Wq = np.asarray(inputs["Wq"], f32)
    Wk = np.asarray(inputs["Wk"], f32)
    Wv = np.asarray(inputs["Wv"], f32)
    Wp = np.asarray(inputs["Wp"], f32)
    bp = np.asarray(inputs["bp"], f32)
    W1 = np.asarray(inputs["W1"], f32)
    b1 = np.asarray(inputs["b1"], f32)
    W2 = np.asarray(inputs["W2"], f32)
    b2 = np.asarray(inputs["b2"], f32)
    g1 = np.asarray(inputs["g1"], f32)
    be1 = np.asarray(inputs["be1"], f32)
    g2 = np.asarray(inputs["g2"], f32)
    be2 = np.asarray(inputs["be2"], f32)

    Wq_f = Wq * g1[None, :, None]
    Wk_f = Wk * g1[None, :, None]
    Wv_f = Wv * g1[None, :, None]
    bq_f = np.einsum("d,hde->he", be1, Wq)
    bk_f = np.einsum("d,hde->he", be1, Wk)
    bv_f = np.einsum("d,hde->he", be1, Wv)
    res_const = bp + bv_f.reshape(H * DH) @ Wp

    W1_f = W1 * g2[:, None]
    b1_f = (b1 + be2 @ W1).astype(f32)

    xall = x.astype(NP_BF16)
    b2bc = np.ascontiguousarray(np.broadcast_to(b2, (128, D))).astype(f32)
    w1_d = np.ascontiguousarray(
        W1_f.reshape(DC, 128, FF).transpose(1, 0, 2).astype(NP_BF16))
    w2_d = np.ascontiguousarray(
        W2.reshape(32, 128, D).transpose(1, 0, 2).astype(NP_BF16))
    wp_r = Wp.reshape(8, 2, 64, D)  # [s, half, 64, D]
    wp0_d = np.ascontiguousarray(wp_r[:, 0].transpose(1, 0, 2).astype(NP_BF16))
    wp1_d = np.ascontiguousarray(wp_r[:, 1].transpose(1, 0, 2).astype(NP_BF16))
    b1p = np.ascontiguousarray(b1_f.reshape(32, 128).T)

    masks = np.zeros((128, 512), f32)
    ii = np.arange(128)[:, None]
    jj = np.arange(256)[None, :]
    masks[:, 0:256] = np.where(ii <= jj, 0.0, -BIG)
    masks[:, 256:512] = np.where(128 + ii <= jj, 0.0, -BIG)
    masks = masks.astype(NP_BF16)
    idm = np.eye(128, dtype=f32).astype(NP_BF16)

    shared = dict(
        xall=xall.view(np.uint16), b2bc=b2bc,
        w1=w1_d.view(np.uint16), w2=w2_d.view(np.uint16),
        wp0=wp0_d.view(np.uint16), wp1=wp1_d.view(np.uint16), b1p=b1p,
        masks=masks.view(np.uint16), idm=idm.view(np.uint16),
    )

    in_maps = []
    for c in range(NCORES):
        h0, h1 = 2 * c, 2 * c + 1
        wq_c = np.concatenate([Wq_f[h0], Wq_f[h1]], axis=1)
        wk_c = np.concatenate([Wk_f[h0], Wk_f[h1]], axis=1)
        wv_c = np.concatenate([Wv_f[h0], Wv_f[h1]], axis=1)
        pack = lambda w: np.ascontiguousarray(
            w.reshape(DC, 128, 128).transpose(1, 0, 2).astype(NP_BF16)).view(np.uint16)
        bqk_c = np.stack(
            [np.concatenate([bq_f[h0], bq_f[h1]]),
             np.concatenate([bk_f[h0], bk_f[h1]])], axis=1).astype(f32)
        xres_c = (x[QL * c:QL * (c + 1)] + res_const).astype(f32)
        in_maps.append(dict(
            shared, wq=pack(wq_c), wk=pack(wk_c), wv=pack(wv_c),
            bqk=np.ascontiguousarray(bqk_c), xres=np.ascontiguousarray(xres_c)))
    return in_maps


def _unshard(results):
    out = np.empty((T, D), np.float32)
    for c in range(NCORES):
        out[QL * c:QL * (c + 1)] = results[c]["out"]
    return out.reshape(B, P, D)


def kernel(**inputs):
    in_maps = _host_pack(inputs)
    nc = _get_nc()
    res = run_bass_kernel_spmd(nc, in_maps, core_ids=list(range(NCORES)))
    return _unshard(res.results)
